# revision 51
# baseline (speedup 1.0000x reference)
"""Trainium2 Bass kernel for a 3-layer ContinuousConv (Open3D-style) point
cloud network + 4-layer FC head.

Strategy (8 NeuronCores, data-parallel over points), bf16 matmul datapath:
  - 10000 points padded to 10240, sharded 1280/core, 10 tiles of 128 points
    (4 PE quadrants x 32 neighbors).
  - Layer 1 is host-encoded: A1[j,m,c] = sum_k S[j,k,m] f4[nb,c] is a pure
    input transformation (Cin=4), computed on host and shipped K-packed
    (32 cells x 4 ch = 128 contraction rows, 7 chunks); on device layer 1 is
    just 7 accumulating matmuls per tile + epilogue. No gather, no S.
  - Layers 2/3: fN gathered by ONE dma_gather per tile (4096 rows of 256B,
    int16 indices, 16-partition wrap replicated x8; ~9 ns/row Q7 desc-gen).
    Activation tables are [*, 128] bf16 rows (256B) in DRAM.
  - Per point j the trilinear scatter matrix S[j] (32 neighbors x 216 cells)
    is built per tile from three 1-D hat functions relu(1-|cell-coord|)
    (computed once) via a broadcast-AP outer product, cast to bf16.
  - Conv layer = per-point matmul A[j]^T = fN[j]^T @ S[j] on the PE (2
    matmuls even/odd cells -> PSUM, explicit tile_position per quadrant;
    psA groups emitted interleaved across quadrants so LDWEIGHTS of the
    next group pulls ahead of in-flight matmuls), PSUM drained fp32->bf16
    split across DVE/ACT, then out[j,:] = sum_t A2[:,j] @ W[t] over 108
    cell-pair steps (contraction 128 = 2 cells x 64 ch) in PSUM.
  - AllGather of the per-core activation slab between conv layers.
  - FC head fused per tile after conv3 (PE transpose + 4 small matmuls).
"""

import os
import numpy as np

DBG = int(os.environ.get("KBUILD_DEBUG", "0"))
DBG_NOGATHER = int(os.environ.get("KBUILD_NOGATHER", "0"))
DBG_TILES = int(os.environ.get("KBUILD_TILES", "0"))
IROT = int(os.environ.get("KBUILD_ROT", "0"))
GQ = int(os.environ.get("KBUILD_GQ", "4"))
GILV = int(os.environ.get("KBUILD_GILV", "1"))

# ---------------------------------------------------------------- constants
N = 10000
K = 32
KS = 6
M = 216          # KS^3
HC = 108         # cell pairs
EXTENT = 3.0
EPS = 1e-12
FOUR_OVER_PI = float(4.0 / np.pi)
BIG = 1.0e6

NCORES = 8
PPC = 1250       # real points per core
PT = 128         # points per tile (4 PE quadrants x 32 neighbors)
NTILES = 10
PPCP = PT * NTILES          # 1280 padded points per core
COLS = NTILES * 32          # 320
NPAD = NCORES * PPCP        # 10752
C = 64           # uniform channel width (padded)
CG = 128         # gather row width (bf16, 256B rows for dma_gather)
NI = 32 * 128    # gathered rows per tile (4096)

_CACHE = {}

# Measured descriptor pairing of the [96,32]-index indirect DMA gather on this
# runtime: dest slot (p,b) receives the index at flat slot GATHER_PERM[p,b] of
# the [96,32] index tile. Host pre-permutes so rows land where intended.
import base64 as _b64
GATHER_PERM = np.frombuffer(
    _b64.b64decode(
        "AAABAAIAAwAEAAUABgAHAAgACQAKAAsADAANAA4ADwAQABEAEgATABQAFQAWABcAGAAZABoAGwAcAB0AHgAfACAAIQAiACMAJAAlACYAJwAoACkAKgArACwALQAuAC8AMAAxADIAMwA0ADUANgA3ADgAOQA6ADsAPAA9AD4APwBAAEEAQgBDAEQARQBGAEcASABJAEoASwBMAE0ATgBPAFAAUQBSAFMAVABVAFYAVwBYAFkAWgBbAFwAXQBeAF8AYABhAGIAYwBkAGUAZgBnAGgAaQBqAGsAbABtAG4AbwBwAHEAcgBzAHQAdQB2AHcAeAB5AHoAewB8AH0AfgB/AIAAgQCCAIMAhACFAIYAhwCIAIkAigCLAIwAjQCOAI8AkACRAJIAkwCUAJUAlgCXAJgAmQCaAJsAnACdAJ4AnwCgAKEAogCjAKQApQCmAKcAqACpAKoAqwCsAK0ArgCvALAAsQCyALMAtAC1ALYAtwC4ALkAugC7ALwAvQC+AL8AwADBAMIAwwDEAMUAxgDHAMgAyQDKAMsAzADNAM4AzwDQANEA0gDTANQA1QDWANcA2ADZANoA2wDcAN0A3gDfAOAA4QDiAOMA5ADlAOYA5wDoAOkA6gDrAOwA7QDuAO8A8ADxAPIA8wD0APUA9gD3APgA+QD6APsA/AD9AP4A/wAAAQEBAgEDAQQBBQEGAQcBCAEJAQoBCwEMAQ0BDgEPARABEQESARMBFAEVARYBFwEYARkBGgEbARwBHQEeAR8BIAEhASIBIwEkASUBJgEnASgBKQEqASsBLAEtAS4BLwEwATEBMgEzATQBNQE2ATcBOAE5AToBOwE8AT0BPgE/AUABQQFCAUMBRAFFAUYBRwFIAUkBSgFLAUwBTQFOAU8BUAFRAVIBUwFUAVUBVgFXAVgBWQFaAVsBXAFdAV4BXwFgAWEBYgFjAWQBZQFmAWcBaAFpAWoBawFsAW0BbgFvAXABcQFyAXMBdAF1AXYBdwF4AXkBegF7AXwBfQF+AX8BgAGBAYIBgwGEAYUBhgGHAYgBiQGKAYsBjAGNAY4BjwGQAZEBkgGTAZQBlQGWAZcBmAGZAZoBmwGcAZ0BngGfAaABoQGiAaMBpAGlAaYBpwGoAakBqgGrAawBrQGuAa8BsAGxAbIBswG0AbUBtgG3AbgBuQG6AbsBvAG9Ab4BvwHAAcEBwgHDAcQBxQHGAccByAHJAcoBywHMAc0BzgHPAdAB0QHSAdMB1AHVAdYB1wHYAdkB2gHbAdwB3QHeAd8B4AHhAeIB4wHkAeUB5gHnAegB6QHqAesB7AHtAe4B7wHwAfEB8gHzAfQB9QH2AfcB+AH5AfoB+wH8Af0B/gH/AQACAQICAgMCBAIFAgYCBwIIAgkCCgILAgwCDQIOAg8CEAIRAhICEwIUAhUCFgIXAhgCGQIaAhsCHAIdAh4CHwIgAiECIgIjAiQCJQImAicCKAIpAioCKwIsAi0CLgIvAjACMQIyAjMCNAI1AjYCNwI4AjkCOgI7AjwCPQI+Aj8CQAJBAkICQwJEAkUCRgJHAkgCSQJKAksCTAJNAk4CTwJQAlECUgJTAlQCVQJWAlcCWAJZAloCWwJcAl0CXgJfAmACYQJiAmMCZAJlAmYCZwJoAmkCagJrAmwCbQJuAm8CcAJxAnICcwJ0AnUCdgJ3AngCeQJ6AnsCfAJ9An4CfwKAAoECggKDAoQChQKGAocCiAKJAooCiwKMAo0CjgKPApACkQKSApMClAKVApYClwKYApkCmgKbApwCnQKeAp8CoAKhAqICowKkAqUCpgKnAqgCqQKqAqsCrAKtAq4CrwKwArECsgKzArQCtQK2ArcCuAK5AroCuwK8Ar0CvgK/AsACwQLCAsMCxALFAsYCxwLIAskCygLLAswCzQLOAs8C0ALRAtIC0wLUAtUC1gLXAtgC2QLaAtsC3ALdAt4C3wLgAuEC4gLjAuQC5QLmAucC6ALpAuoC6wLsAu0C7gLvAvAC8QLyAvMC9AL1AvYC9wL4AvkC+gL7AvwC/QL+Av8CAAMBAwIDAwMEAwUDBgMHAwgDCQMKAwsDDAMNAw4DDwMQAxEDEgMTAxQDFQMWAxcDGAMZAxoDGwMcAx0DHgMfAyADIQMiAyMDJAMlAyYDJwMoAykDKgMrAywDLQMuAy8DMAMxAzIDMwM0AzUDNgM3AzgDOQM6AzsDPAM9Az4DPwNAA0EDQgNDA0QDRQNGA0cDSANJA0oDSwNMA00DTgNPA1ADUQNSA1MDVANVA1YDVwNYA1kDWgNbA1wDXQNeA18DYANhA2IDYwNkA2UDZgNnA2gDaQNqA2sDbANtA24DbwNwA3EDcgNzA3QDdQN2A3cDeAN5A3oDewN8A30DfgN/A4ADgQOCA4MDhAOFA4YDhwOIA4kDigOLA4wDjQOOA48DkAORA5IDkwOUA5UDlgOXA5gDmQOaA5sDnAOdA54DnwOgA6EDogOjA6QDpQOmA6cDqAOpA6oDqwOsA60DrgOvA7ADsQOyA7MDtAO1A7YDtwO4A7kDugO7A7wDvQO+A78DwAPBA8IDwwPEA8UDxgPHA8gDyQPKA8sDzAPNA84DzwPQA9ED0gPTA9QD1QPWA9cD2APZA9oD2wPcA90D3gPfA+AD4QPiA+MD5APlA+YD5wPoA+kD6gPrA+wD7QPuA+8D8APxA/ID8wP0A/UD9gP3A/gD+QP6A/sD/AP9A/4D/wMABAEEAgQDBAQEBQQGBAcECAQJBAoECwQMBA0EDgQPBBAEEQQSBBMEFAQVBBYEFwQYBBkEGgQbBBwEHQQeBB8EIAQhBCIEIwQkBCUEJgQnBCgEKQQqBCsELAQtBC4ELwQwBDEEMgQzBDQENQQ2BDcEOAQ5BDoEOwQ8BD0EPgQ/BEAEQQRCBEMERARFBEYERwRIBEkESgRLBEwETQROBE8EUARRBFIEUwRUBFUEVgRXBFgEWQRaBFsEXARdBF4EXwRgBGEEYgRjBGQEZQRmBGcEaARpBGoEawRsBG0EbgRvBHAEcQRyBHMEdAR1BHYEdwR4BHkEegR7BHwEfQR+BH8EgASBBIIEgwSEBIUEhgSHBIgEiQSKBIsEjASNBI4EjwSQBJEEkgSTBJQElQSWBJcEmASZBJoEmwScBJ0EngSfBKAEoQSiBKMEpASlBKYEpwSoBKkEqgSrBKwErQSuBK8EsASxBLIEswS0BLUEtgS3BLgEuQS6BLsEvAS9BL4EvwTABMEEwgTDBMQExQTGBMcEyATJBMoEywTMBM0EzgTPBNAE0QTSBNME1ATVBNYE1wTYBNkE2gTbBNwE3QTeBN8E4AThBOIE4wTkBOUE5gTnBOgE6QTqBOsE7ATtBO4E7wTwBPEE8gTzBPQE9QT2BPcE+AT5BPoE+wT8BP0E/gT/BAAFAQUCBQMFBAUFBQYFBwUIBQkFCgULBQwFDQUOBQ8FEAURBRIFEwUUBRUFFgUXBRgFGQUaBRsFHAUdBR4FHwUgBSEFIgUjBSQFJQUmBScFKAUpBSoFKwUsBS0FLgUvBTAFMQUyBTMFNAU1BTYFNwU4BTkFOgU7BTwFPQU+BT8FQAVBBUIFQwVEBUUFRgVHBUgFSQVKBUsFTAVNBU4FTwVQBVEFUgVTBVQFVQVWBVcFWAVZBVoFWwVcBV0FXgVfBWAFYQViBWMFZAVlBWYFZwVoBWkFagVrBWwFbQVuBW8FcAVxBXIFcwV0BXUFdgV3BXgFeQV6BXsFfAV9BX4FfwWABYEFggWDBYQFhQWGBYcFiAWJBYoFiwWMBY0FjgWPBZAFkQWSBZMFlAWVBZYFlwWYBZkFmgWbBZwFnQWeBZ8FoAWhBaIFowWkBaUFpgWnBagFqQWqBasFrAWtBa4FrwWwBbEFsgWzBbQFtQW2BbcFuAW5BboFuwW8Bb0FvgW/BcAFwQXCBcMFxAXFBcYFxwXIBckFygXLBcwFzQXOBc8F0AXRBdIF0wXUBdUF1gXXBdgF2QXaBdsF3AXdBd4F3wXgBeEF4gXjBeQF5QXmBecF6AXpBeoF6wXsBe0F7gXvBfAF8QXyBfMF9AX1BfYF9wX4BfkF+gX7BfwF/QX+Bf8FAAYBBgIGAwYEBgUGBgYHBggGCQYKBgsGDAYNBg4GDwYQBhEGEgYTBhQGFQYWBhcGGAYZBhoGGwYcBh0GHgYfBiAGIQYiBiMGJAYlBiYGJwYoBikGKgYrBiwGLQYuBi8GMAYxBjIGMwY0BjUGNgY3BjgGOQY6BjsGPAY9Bj4GPwZABkEGQgZDBkQGRQZGBkcGSAZJBkoGSwZMBk0GTgZPBlAGUQZSBlMGVAZVBlYGVwZYBlkGWgZbBlwGXQZeBl8GYAZhBmIGYwZkBmUGZgZnBmgGaQZqBmsGbAZtBm4GbwZwBnEGcgZzBnQGdQZ2BncGeAZ5BnoGewZ8Bn0GfgZ/BoAGgQaCBoMGhAaFBoYGhwaIBokGigaLBowGjQaOBo8GkAaRBpIGkwaUBpUGlgaXBpgGmQaaBpsGnAadBp4GnwagBqEGogajBqQGpQamBqcGqAapBqoGqwasBq0GrgavBrAGsQayBrMGtAa1BrYGtwa4BrkGuga7BrwGvQa+Br8GwAbBBsIGwwbEBsUGxgbHBsgGyQbKBssGzAbNBs4GzwbQBtEG0gbTBtQG1QbWBtcG2AbZBtoG2wbcBt0G3gbfBuAG4QbiBuMG5AblBuYG5wboBukG6gbrBuwG7QbuBu8G8AbxBvIG8wb0BvUG9gb3BvgG+Qb6BvsG/Ab9Bv4G/wYABwEHAgcDBwQHBQcGBwcHCAcJBwoHCwcMBw0HDgcPBxAHEQcSBxMHFAcVBxYHFwcYBxkHGgcbBxwHHQceBx8HIAchByIHIwckByUHJgcnBygHKQcqBysHLActBy4HLwcwBzEHMgczBzQHNQc2BzcHOAc5BzoHOwc8Bz0HPgc/B0AHQQdCB0MHRAdFB0YHRwdIB0kHSgdLB0wHTQdOB08HUAdRB1IHUwdUB1UHVgdXB1gHWQdaB1sHXAddB14HXwdgB2EHYgdjB2QHZQdmB2cHaAdpB2oHawdsB20HbgdvB3AHcQdyB3MHdAd1B3YHdwd4B3kHegd7B3wHfQd+B38HgAeBB4IHgweEB4UHhgeHB4gHiQeKB4sHjAeNB44HjweQB5EHkgeTB5QHlQeWB5cHmAeZB5oHmwecB50HngefB6AHoQeiB6MHpAelB6YHpweoB6kHqgerB6wHrQeuB68HsAexB7IHswe0B7UHtge3B7gHuQe6B7sHvAe9B74HvwfAB8EHwgfDB8QHxQfGB8cHyAfJB8oHywfMB80HzgfPB9AH0QfSB9MH1AfVB9YH1wfYB9kH2gfbB9wH3QfeB98H4AfhB+IH4wfkB+UH5gfnB+gH6QfqB+sH7AftB+4H7wfwB/EH8gfzB/QH9Qf2B/cH+Af5B/oH+wf8B/0H/gf/BwAIAQgCCAMIBAgFCAYIBwgICAkICggLCAwIDQgOCA8IEAgRCBIIEwgUCBUIFggXCBgIGQgaCBsIHAgdCB4IHwggCCEIIggjCCQIJQgmCCcIKAgpCCoIKwgsCC0ILggvCDAIMQgyCDMINAg1CDYINwg4CDkIOgg7CDwIPQg+CD8IQAhBCEIIQwhECEUIRghHCEgISQhKCEsITAhNCE4ITwhQCFEIUghTCFQIVQhWCFcIWAhZCFoIWwhcCF0IXghfCGAIYQhiCGMIZAhlCGYIZwhoCGkIaghrCGwIbQhuCG8IcAhxCHIIcwh0CHUIdgh3CHgIeQh6CHsIfAh9CH4IfwiACIEIggiDCIQIhQiGCIcIiAiJCIoIiwiMCI0IjgiPCJAIkQiSCJMIlAiVCJYIlwiYCJkImgibCJwInQieCJ8IoAihCKIIowikCKUIpginCKgIqQiqCKsIrAitCK4IrwiwCLEIsgizCLQItQi2CLcIuAi5CLoIuwi8CL0Ivgi/CMAIwQjCCMMIxAjFCMYIxwjICMkIygjLCMwIzQjOCM8I0AjRCNII0wjUCNUI1gjXCNgI2QjaCNsI3AjdCN4I3wjgCOEI4gjjCOQI5QjmCOcI6AjpCOoI6wjsCO0I7gjvCPAI8QjyCPMI9Aj1CPYI9wj4CPkI+gj7CPwI/Qj+CP8IAAkBCQIJAwkECQUJBgkHCQgJCQkKCQsJDAkNCQ4JDwkQCREJEgkTCRQJFQkWCRcJGAkZCRoJGwkcCR0JHgkfCSAJIQkiCSMJJAklCSYJJwkoCSkJKgkrCSwJLQkuCS8JMAkxCTIJMwk0CTUJNgk3CTgJOQk6CTsJPAk9CT4JPwlACUEJQglDCUQJRQlGCUcJSAlJCUoJSwlMCU0JTglPCVAJUQlSCVMJVAlVCVYJVwlYCVkJWglbCVwJXQleCV8JYAlhCWIJYwlkCWUJZglnCWgJaQlqCWsJbAltCW4JbwlwCXEJcglzCXQJdQl2CXcJeAl5CXoJewl8CX0Jfgl/CYAJgQmCCYMJhAmFCYYJhwmICYkJigmLCYwJjQmOCY8JkAmRCZIJkwmUCZUJlgmXCZgJmQmaCZsJnAmdCZ4JnwmgCaEJogmjCaQJpQmmCacJqAmpCaoJqwmsCa0JrgmvCbAJsQmyCbMJtAm1CbYJtwm4CbkJugm7CbwJvQm+Cb8JwAnBCcIJwwnECcUJxgnHCcgJyQnKCcsJzAnNCc4JzwnQCdEJ0gnTCdQJ1QnWCdcJ2AnZCdoJ2wncCd0J3gnfCeAJ4QniCeMJ5AnlCeYJ5wnoCekJ6gnrCewJ7QnuCe8J8AnxCfIJ8wn0CfUJ9gn3CfgJ+Qn6CfsJ/An9Cf4J/wkACgEKAgoDCgQKBQoGCgcKCAoJCgoKCwoMCg0KDgoPChAKEQoSChMKFAoVChYKFwoYChkKGgobChwKHQoeCh8KIAohCiIKIwokCiUKJgonCigKKQoqCisKLAotCi4KLwowCjEKMgozCjQKNQo2CjcKOAo5CjoKOwo8Cj0KPgo/CkAKQQpCCkMKRApFCkYKRwpICkkKSgpLCkwKTQpOCk8KUApRClIKUwpUClUKVgpXClgKWQpaClsKXApdCl4KXwpgCmEKYgpjCmQKZQpmCmcKaAppCmoKawpsCm0KbgpvCnAKcQpyCnMKdAp1CnYKdwp4CnkKegp7CnwKfQp+Cn8KgAqBCoIKgwqECoUKhgqHCogKiQqKCosKjAqNCo4KjwqQCpEKkgqTCpQKlQqWCpcKmAqZCpoKmwqcCp0KngqfCqAKoQqiCqMKpAqlCqYKpwqoCqkKqgqrCqwKrQquCq8KsAqxCrIKswq0CrUKtgq3CrgKuQq6CrsKvAq9Cr4KvwrACsEKwgrDCsQKxQrGCscKyArJCsoKywrMCs0KzgrPCtAK0QrSCtMK1ArVCtYK1wrYCtkK2grbCtwK3QreCt8K4ArhCuIK4wrkCuUK5grnCugK6QrqCusK7ArtCu4K7wrwCvEK8grzCvQK9Qr2CvcK+Ar5CvoK+wr8Cv0K/gr/CgALAQsCCwMLBAsFCwYLBwsICwkLCgsLCwwLDQsOCw8LEAsRCxILEwsUCxULFgsXCxgLGQsaCxsLHAsdCx4LHwsgCyELIgsjCyQLJQsmCycLKAspCyoLKwssCy0LLgsvCzALMQsyCzMLNAs1CzYLNws4CzkLOgs7CzwLPQs+Cz8LQAtBC0ILQwtEC0ULRgtHC0gLSQtKC0sLTAtNC04LTwtQC1ELUgtTC1QLVQtWC1cLWAtZC1oLWwtcC10LXgtfC2ALYQtiC2MLZAtlC2YLZwtoC2kLagtrC2wLbQtuC28LcAtxC3ILcwt0C3ULdgt3C3gLeQt6C3sLfAt9C34LfwuAC4ELgguDC4QLhQuGC4cLiAuJC4oLiwuMC40LjguPC5ALkQuSC5MLlAuVC5YLlwuYC5kLmgubC5wLnQueC58LoAuhC6ILowukC6ULpgunC6gLqQuqC6sLrAutC64LrwuwC7ELsguzC7QLtQu2C7cLuAu5C7oLuwu8C70Lvgu/C8ALwQvCC8MLxAvFC8YLxwvIC8kLygvLC8wLzQvOC88L0AvRC9IL0wvUC9UL1gvXC9gL2QvaC9sL3AvdC94L3wvgC+EL4gvjC+QL5QvmC+cL6AvpC+oL6wvsC+0L7gvvC/AL8QvyC/ML9Av1C/YL9wv4C/kL+gv7C/wL/Qv+C/8L"
    ), dtype=np.int16).astype(np.int64).reshape(96, 32)

NOPERM = int(os.environ.get("KBUILD_NOPERM", "0"))


def _permute_idx_block(blk):
    """blk [96,32] desired per-slot indices -> send layout for the HW."""
    if NOPERM:
        return blk
    out = np.empty_like(blk)
    out.reshape(-1)[GATHER_PERM.reshape(-1)] = blk.reshape(-1)
    return out



# ---------------------------------------------------------------- bass build
def _build_program():
    import concourse.bass as bass
    import concourse.tile as tile
    from concourse import mybir, bacc
    from concourse.masks import make_identity
    from contextlib import ExitStack

    f32 = mybir.dt.float32
    bf16 = mybir.dt.bfloat16
    i32 = mybir.dt.int32
    Alu = mybir.AluOpType
    Act = mybir.ActivationFunctionType

    nc = bacc.Bacc("TRN2", target_bir_lowering=False, debug=False,
                   num_devices=NCORES, num_swdge_queues=GQ)

    # ---- I/O ----
    i16 = mybir.dt.int16
    nidx_d = nc.dram_tensor("nidx", [128, (NI // 16) * NTILES], i16,
                            kind="ExternalInput")
    uin = nc.dram_tensor("uin", [PT, 3 * COLS], f32, kind="ExternalInput")
    cnti_d = nc.dram_tensor("cntinv", [PT, NTILES], f32, kind="ExternalInput")
    # layer 1 host-encoded: A1 in K-packed layout (32 cells x 4ch rows)
    a1_d = nc.dram_tensor("a1", [128, NTILES * 7 * PT], bf16,
                          kind="ExternalInput")
    w1_d = nc.dram_tensor("w1", [128, 7 * C], bf16, kind="ExternalInput")
    w_d = [nc.dram_tensor(f"w{l}", [128, HC * C], bf16, kind="ExternalInput")
           for l in (2, 3)]
    bias_d = [nc.dram_tensor(f"bias{l}", [PT, C], f32, kind="ExternalInput")
              for l in (1, 2, 3)]
    iota_d = nc.dram_tensor("iota6", [PT, 6], f32, kind="ExternalInput")
    wfc_d = [nc.dram_tensor(f"wfc{l}", [64, 64], bf16, kind="ExternalInput")
             for l in (1, 2, 3)]
    wout_d = nc.dram_tensor("wout", [64, 8], bf16, kind="ExternalInput")
    bfc_d = [nc.dram_tensor(f"bfc{l}", [64, 1], f32, kind="ExternalInput")
             for l in (1, 2, 3)]
    bout_d = nc.dram_tensor("bout", [8, 1], f32, kind="ExternalInput")
    if DBG != 1:
        outT = nc.dram_tensor("outT", [3, PPCP], f32, kind="ExternalOutput")
    else:
        outT = None

    # internal DRAM
    xl1_kind = "ExternalOutput" if DBG else "Internal"
    xloc = [nc.dram_tensor("xloc1", [PPCP, CG], bf16, kind=xl1_kind),
            nc.dram_tensor("xloc2", [PPCP, CG], bf16,
                           kind="Internal" if DBG == 1 else xl1_kind)]
    xfull = [nc.dram_tensor(f"xfull{l}", [NPAD, CG], bf16, addr_space="Shared")
             for l in (1, 2)]
    warm_in = nc.dram_tensor("warm_in", [128, 1], f32, kind="Internal")
    warm_out = nc.dram_tensor("warm_out", [128 * NCORES, 1], f32,
                              addr_space="Shared")

    with tile.TileContext(nc) as tc, ExitStack() as stk:
        # ---------- persistent small constants ----------
        cpool = stk.enter_context(tc.tile_pool(name="const", bufs=1))
        cnti_sb = cpool.tile([PT, NTILES], f32)
        nc.sync.dma_start(out=cnti_sb[:], in_=cnti_d[:, :])
        iota_sb = cpool.tile([PT, 6], f32)
        nc.sync.dma_start(out=iota_sb[:], in_=iota_d[:, :])
        bias_sb = []
        for l in range(3):
            b = cpool.tile([PT, C], f32, name=f"biassb{l}")
            nc.sync.dma_start(out=b[:], in_=bias_d[l][:, :])
            bias_sb.append(b)
        wfc_sb = []
        for l in range(3):
            w = cpool.tile([64, 64], bf16, name=f"wfcsb{l}")
            nc.sync.dma_start(out=w[:], in_=wfc_d[l][:, :])
            wfc_sb.append(w)
        wout_sb = cpool.tile([64, 8], bf16)
        nc.sync.dma_start(out=wout_sb[:], in_=wout_d[:, :])
        bfc_sb = []
        for l in range(3):
            b = cpool.tile([64, 1], f32, name=f"bfcsb{l}")
            nc.sync.dma_start(out=b[:], in_=bfc_d[l][:, :])
            bfc_sb.append(b)
        bout_sb = cpool.tile([8, 1], f32)
        nc.sync.dma_start(out=bout_sb[:], in_=bout_d[:, :])
        ident_sb = cpool.tile([PT, PT], bf16)
        make_identity(nc, ident_sb[:])

        # gather indices for all tiles (int16, 16-part wrap replicated x8)
        nidx16_sb = cpool.tile([128, (NI // 16) * NTILES], mybir.dt.int16)
        nc.sync.dma_start(out=nidx16_sb[:], in_=nidx_d[:, :])

        # layer-1 host-encoded A1 + packed W1
        a1_sb = cpool.tile([128, NTILES * 7 * PT], bf16, name="a1sb")
        nc.sync.dma_start(out=a1_sb[:], in_=a1_d[:, :])
        w1_sb = cpool.tile([128, 7 * C], bf16, name="w1sb")
        nc.sync.dma_start(out=w1_sb[:], in_=w1_d[:, :])

        # zero the channel-pad region of the internal activation tables once
        # tiny AllGather first: pays the CC warmup + entry barrier inside
        # the geometry/layer-1 window instead of on the critical path
        if DBG != 1:
            nc.gpsimd.collective_compute(
                "AllGather", Alu.bypass,
                replica_groups=[list(range(NCORES))],
                ins=[warm_in.ap().opt()],
                outs=[warm_out.ap().opt()],
            )
        zpad = cpool.tile([128, CG - C], bf16, name="zpad")
        nc.vector.memset(zpad[:], 0.0)
        for xl in xloc:
            for r0 in range(0, PPCP, 128):
                rn = min(128, PPCP - r0)
                nc.sync.dma_start(out=xl[r0:r0 + rn, C:CG],
                                  in_=zpad[0:rn, :])

        # hats: per (j,k) pair the 6-cell 1-D trilinear weights, per dim
        hat_sb = [cpool.tile([PT, 6 * COLS], f32, name=f"hat{d}")
                  for d in range(3)]

        # ---------- geometry (ball_to_cube -> grid coords -> hats) ----------
        with tc.tile_pool(name="geo", bufs=1) as geo:
            def gt(tag):
                return geo.tile([PT, COLS], f32, name=tag)

            V = nc.vector
            S_ = nc.scalar

            x = gt("gx"); y = gt("gy"); z = gt("gz")
            nc.sync.dma_start(out=x[:], in_=uin[:, 0:COLS])
            nc.sync.dma_start(out=y[:], in_=uin[:, COLS:2 * COLS])
            nc.sync.dma_start(out=z[:], in_=uin[:, 2 * COLS:3 * COLS])

            u8 = mybir.dt.uint8
            cone_m = geo.tile([PT, COLS], u8, name="cone_m")
            xmaj_m = geo.tile([PT, COLS], u8, name="xmaj_m")
            den_m = geo.tile([PT, COLS], u8, name="den_m")
            ones = gt("ones")
            nc.vector.memset(ones[:], 1.0)

            xx = gt("xx"); yy = gt("yy"); zz = gt("zz")
            V.tensor_mul(xx[:], x[:], x[:])
            V.tensor_mul(yy[:], y[:], y[:])
            V.tensor_mul(zz[:], z[:], z[:])
            rho2 = gt("rho2"); sq = gt("sq")
            V.tensor_add(rho2[:], xx[:], yy[:])
            V.tensor_add(sq[:], rho2[:], zz[:])
            t0 = gt("t0"); norm = gt("norm")
            V.tensor_scalar_max(t0[:], sq[:], EPS)
            S_.activation(norm[:], t0[:], Act.Sqrt)            # norm
            az = gt("az")
            S_.activation(az[:], z[:], Act.Abs)
            den = gt("den")
            V.tensor_add(den[:], norm[:], az[:])
            rden = gt("rden")
            V.reciprocal(rden[:], den[:])
            t1 = gt("t1")
            V.tensor_scalar_mul(t1[:], norm[:], 3.0)
            V.tensor_mul(t1[:], t1[:], rden[:])                # 3n/(n+|z|)
            s1 = gt("s1")
            S_.activation(s1[:], t1[:], Act.Sqrt)
            V.tensor_scalar_max(t0[:], rho2[:], EPS)
            rr = gt("rr")
            V.reciprocal(rr[:], t0[:])
            S_.activation(rr[:], rr[:], Act.Sqrt)              # 1/sqrt(rho2)
            s2 = gt("s2")
            V.tensor_mul(s2[:], norm[:], rr[:])
            cone = gt("cone")
            V.tensor_scalar_mul(cone[:], zz[:], 1.25)
            V.tensor_tensor(cone_m[:], cone[:], rho2[:], op=Alu.is_gt)
            s = gt("s")
            V.select(s[:], cone_m[:], s1[:], s2[:])
            xc = gt("xc"); yc = gt("yc"); zc = gt("zc")
            V.tensor_mul(xc[:], x[:], s[:])
            V.tensor_mul(yc[:], y[:], s[:])
            sgn = gt("sgn")
            S_.activation(sgn[:], z[:], Act.Sign)
            V.tensor_mul(sgn[:], sgn[:], norm[:])              # sign(z)*norm
            t2 = gt("t2")
            V.tensor_scalar_mul(t2[:], z[:], 1.5)
            V.select(zc[:], cone_m[:], sgn[:], t2[:])
            tm = gt("tm")
            V.tensor_scalar(tm[:], sq[:], EPS, None, op0=Alu.is_ge)
            V.tensor_mul(xc[:], xc[:], tm[:])
            V.tensor_mul(yc[:], yc[:], tm[:])
            V.tensor_mul(zc[:], zc[:], tm[:])

            # cylinder -> cube (xy disc)
            V.tensor_mul(xx[:], xc[:], xc[:])
            V.tensor_mul(yy[:], yc[:], yc[:])
            sqxy = gt("sqxy")
            V.tensor_add(sqxy[:], xx[:], yy[:])
            V.tensor_scalar_max(t0[:], sqxy[:], EPS)
            nxy = gt("nxy")
            S_.activation(nxy[:], t0[:], Act.Sqrt)
            axc = gt("axc"); ayc = gt("ayc")
            S_.activation(axc[:], xc[:], Act.Abs)
            S_.activation(ayc[:], yc[:], Act.Abs)
            V.tensor_tensor(xmaj_m[:], ayc[:], axc[:], op=Alu.is_le)
            sgx = gt("sgx"); sgy = gt("sgy")
            S_.activation(sgx[:], xc[:], Act.Sign)
            S_.activation(sgy[:], yc[:], Act.Sign)
            tx = gt("txv"); ty = gt("tyv")
            V.tensor_mul(tx[:], sgx[:], nxy[:])
            V.tensor_mul(ty[:], sgy[:], nxy[:])
            # safe denominators
            V.tensor_scalar(den_m[:], axc[:], EPS, None, op0=Alu.is_lt)
            xd = gt("xd")
            V.select(xd[:], den_m[:], ones[:], xc[:])
            V.tensor_scalar(den_m[:], ayc[:], EPS, None, op0=Alu.is_lt)
            yd = gt("yd")
            V.select(yd[:], den_m[:], ones[:], yc[:])
            V.reciprocal(t1[:], yd[:])
            V.tensor_mul(t1[:], xc[:], t1[:])
            V.tensor_scalar(t1[:], t1[:], 1.0, -1.0, op0=Alu.min,
                            op1=Alu.max)           # clamp unused branch
            at1 = gt("at1")
            S_.activation(at1[:], t1[:], Act.Arctan)
            V.reciprocal(t2[:], xd[:])
            V.tensor_mul(t2[:], yc[:], t2[:])
            V.tensor_scalar(t2[:], t2[:], 1.0, -1.0, op0=Alu.min,
                            op1=Alu.max)
            at2 = gt("at2")
            S_.activation(at2[:], t2[:], Act.Arctan)
            # xq
            V.tensor_mul(t1[:], ty[:], at1[:])
            V.tensor_scalar_mul(t1[:], t1[:], FOUR_OVER_PI)
            xq = gt("xq")
            V.select(xq[:], xmaj_m[:], tx[:], t1[:])
            # yq
            V.tensor_mul(t2[:], tx[:], at2[:])
            V.tensor_scalar_mul(t2[:], t2[:], FOUR_OVER_PI)
            yq = gt("yq")
            V.select(yq[:], xmaj_m[:], t2[:], ty[:])
            V.tensor_scalar(tm[:], sqxy[:], EPS, None, op0=Alu.is_ge)
            V.tensor_mul(xq[:], xq[:], tm[:])
            V.tensor_mul(yq[:], yq[:], tm[:])

            # grid coords (align_corners): (c+1)*2.5
            coords = []
            for src, tag in ((xq, "ccx"), (yq, "ccy"), (zc, "ccz")):
                cd = gt(tag)
                V.tensor_scalar(cd[:], src[:], 1.0, 2.5, op0=Alu.add,
                                op1=Alu.mult)
                coords.append(cd)

            # hats: w[p, col*6+m] = relu(1 - |iota6[m] - coord[p,col]|)
            iap = iota_sb[:]
            for d in range(3):
                cap = coords[d][:]
                hat = hat_sb[d]
                io_b = bass.AP(iap.tensor, iap.offset,
                               [iap.ap[0], [0, COLS], [1, 6]])
                cd_b = bass.AP(cap.tensor, cap.offset,
                               [cap.ap[0], [1, COLS], [0, 6]])
                V.tensor_tensor(hat[:], io_b, cd_b, op=Alu.subtract)
                S_.activation(hat[:], hat[:], Act.Abs)
                S_.activation(hat[:], hat[:], Act.Relu,
                              bias=1.0, scale=-1.0)              # relu(1-|d|)

        # ---------- conv layers ----------
        wpool = stk.enter_context(tc.tile_pool(name="wpool", bufs=1))
        fnpool = stk.enter_context(tc.tile_pool(name="fn", bufs=2))
        fnbpool = stk.enter_context(tc.tile_pool(name="fnb", bufs=3))
        wyzpool = stk.enter_context(tc.tile_pool(name="wyz", bufs=2))
        spool = stk.enter_context(tc.tile_pool(name="spool", bufs=2))
        apool = stk.enter_context(tc.tile_pool(name="apool", bufs=2))
        xpool = stk.enter_context(tc.tile_pool(name="xpool", bufs=2))
        psA = stk.enter_context(tc.tile_pool(name="psA", bufs=3, space="PSUM"))
        psO = stk.enter_context(tc.tile_pool(name="psO", bufs=2, space="PSUM"))
        psF = stk.enter_context(tc.tile_pool(name="psF", bufs=2, space="PSUM"))
        psG = stk.enter_context(tc.tile_pool(name="psG", bufs=1, space="PSUM"))

        def epilogue(li, t, po, last):
            # relu(out*cntinv + bias) -> xt; write or FC head
            xt = xpool.tile([PT, C], bf16, tag="xt", name=f"xt{li}_{t}")
            nc.vector.scalar_tensor_tensor(
                xt[:], po[:], cnti_sb[:, t:t + 1], bias_sb[li][:],
                op0=Alu.mult, op1=Alu.add)
            nc.scalar.activation(xt[:], xt[:], Act.Relu)
            return xt

        def layer1():
            for t in range(NTILES):
                po = psO.tile([PT, C], f32, tag="psO", name=f"psO0_{t}")
                for k7 in range(7):
                    nc.tensor.matmul(
                        po[:], a1_sb[:, (t * 7 + k7) * PT:(t * 7 + k7 + 1) * PT],
                        w1_sb[:, k7 * C:(k7 + 1) * C],
                        start=(k7 == 0), stop=(k7 == 6))
                xt = epilogue(0, t, po, False)
                nc.sync.dma_start(out=xloc[0][t * PT:(t + 1) * PT, 0:C],
                                  in_=xt[:])
            if DBG != 1:
                nc.gpsimd.collective_compute(
                    "AllGather", Alu.bypass,
                    replica_groups=[list(range(NCORES))],
                    ins=[xloc[0].ap().opt()],
                    outs=[xfull[0].ap().opt()],
                )

        def conv_layer(li, xsrc, xdst):
            wsb = wpool.tile([128, HC * C], bf16, tag="W", name=f"wsb{li}")
            nc.sync.dma_start(out=wsb[:], in_=w_d[li - 1][:, :])
            last = xdst is None
            for t in range(DBG_TILES or NTILES):
                # gather fN: ONE dma_gather per tile (4096 rows of 256B):
                # fball[p, b*CG:(b+1)*CG] = xsrc[flat[b*128+p]]
                fball = fnbpool.tile([128, 32 * CG], bf16, tag="fnb",
                                     name=f"fnb{li}_{t}")
                if DBG_NOGATHER:
                    nc.vector.memset(fball[:], 0.01)
                else:
                    fap = fball[:]
                    # first tile: 4 chunked gathers so stage-1 starts after
                    # the first quarter's desc-gen instead of the full 36us
                    nch = 8 if t == 0 else (2 if t in (1, 2) else 1)
                    nic = NI // nch
                    for j in range(nch):
                        out3 = bass.AP(fap.tensor,
                                       fap.offset + j * (32 // nch) * CG,
                                       [fap.ap[0], [CG, 32 // nch], [1, CG]])
                        nc.gpsimd.dma_gather(
                            out_ap=out3, in_ap=xsrc[:, :],
                            idxs_ap=nidx16_sb[:, t * (NI // 16) + j * (nic // 16):
                                              t * (NI // 16) + (j + 1) * (nic // 16)],
                            num_idxs=nic, num_idxs_reg=nic, elem_size=CG,
                            single_packet=False, queue_num=t % GQ)

                # S tile: [96, 32*216]; col block b holds point r=b*3+q at
                # partitions q*32..q*32+32 (k), cells m = mx*36+my*6+mz
                wyz = wyzpool.tile([PT, 32 * 36], f32, tag="wyz",
                                   name=f"wyz{li}_{t}")
                hy = hat_sb[1][:]
                hz = hat_sb[2][:]
                hy_b = bass.AP(hy.tensor, hy.offset + t * 192,
                               [hy.ap[0], [6, 32], [1, 6], [0, 6]])
                hz_b = bass.AP(hz.tensor, hz.offset + t * 192,
                               [hz.ap[0], [6, 32], [0, 6], [1, 6]])
                nc.vector.tensor_tensor(wyz[:], hy_b, hz_b, op=Alu.mult)
                st = spool.tile([PT, 32 * M], bf16, tag="S", name=f"st{li}_{t}")
                hx = hat_sb[0][:]
                wz = wyz[:]
                hx_b = bass.AP(hx.tensor, hx.offset + t * 192,
                               [hx.ap[0], [6, 32], [1, 6], [0, 36]])
                wz_b = bass.AP(wz.tensor, wz.offset,
                               [wz.ap[0], [36, 32], [0, 6], [1, 36]])
                nc.vector.tensor_tensor(st[:], hx_b, wz_b, op=Alu.mult)

                # stage 1: per-point A^T, packed 4 points per PSUM tile
                a2 = apool.tile([128, PT * HC], bf16, tag="A2",
                                name=f"a2_{li}_{t}")
                gorder = ([g0 + j * 8 for g0 in range(8) for j in range(4)]
                          if GILV else list(range(PT // 4)))
                for g in gorder:
                    ps = psA.tile([128, 4 * HC], f32, tag="psA",
                                  name=f"psA{li}_{t}_{g}")
                    for w_ in range(4):
                        r = g * 4 + w_
                        q = r // 32 if IROT == 0 else r % 4
                        b = r % 32 if IROT == 0 else r // 4
                        fsl = fball[q * 32:(q + 1) * 32, b * CG:b * CG + C]
                        sbase = st[q * 32:(q + 1) * 32, b * M:(b + 1) * M]
                        s_ev = bass.AP(sbase.tensor, sbase.offset,
                                       [sbase.ap[0], [2, HC]])
                        s_od = bass.AP(sbase.tensor, sbase.offset + 1,
                                       [sbase.ap[0], [2, HC]])
                        nc.tensor.matmul(ps[0:64, w_ * HC:(w_ + 1) * HC],
                                         fsl, s_ev, start=True, stop=True,
                                         tile_position=(q * 32, 0))
                        nc.tensor.matmul(ps[64:128, w_ * HC:(w_ + 1) * HC],
                                         fsl, s_od, start=True, stop=True,
                                         tile_position=(q * 32, 64))
                    dst = a2[:, g * 4 * HC:(g + 1) * 4 * HC]
                    if g % 2 == 1:
                        nc.scalar.copy(dst, ps[:])
                    else:
                        nc.vector.tensor_copy(dst, ps[:])

                # stage 2: accumulate over 108 cell pairs
                po = psO.tile([PT, C], f32, tag="psO", name=f"psO{li}_{t}")
                a2ap = a2[:]
                for ts_ in range(HC):
                    lhs = bass.AP(a2ap.tensor, a2ap.offset + ts_,
                                  [a2ap.ap[0], [HC, PT]])
                    nc.tensor.matmul(po[:], lhs,
                                     wsb[:, ts_ * C:(ts_ + 1) * C],
                                     start=(ts_ == 0), stop=(ts_ == HC - 1))

                xt = epilogue(li, t, po, last)

                if not last:
                    nc.sync.dma_start(out=xdst[t * PT:(t + 1) * PT, 0:C],
                                      in_=xt[:])
                else:
                    # FC head fused per tile
                    pt_ = psF.tile([64, PT], bf16, tag="psT",
                                   name=f"psT{t}")
                    nc.tensor.transpose(pt_[:], xt[:], ident_sb[:])
                    h = xpool.tile([64, PT], bf16, tag="h0", name=f"h0_{t}")
                    nc.vector.tensor_copy(h[:], pt_[:])
                    for l in range(3):
                        pf = psF.tile([64, PT], f32, tag="psT",
                                      name=f"psf{t}_{l}")
                        nc.tensor.matmul(pf[:], wfc_sb[l][:], h[:],
                                         start=True, stop=True)
                        h = xpool.tile([64, PT], bf16, tag=f"h{l + 1}",
                                       name=f"h{l + 1}_{t}")
                        nc.scalar.activation(h[:], pf[:], Act.Relu,
                                             bias=bfc_sb[l][:])
                    pg = psG.tile([8, PT], f32, tag="psG", name=f"psG{t}")
                    nc.tensor.matmul(pg[:], wout_sb[:], h[:],
                                     start=True, stop=True)
                    ot = xpool.tile([8, PT], f32, tag="ot", name=f"ot{t}")
                    nc.vector.tensor_scalar(ot[:], pg[:], bout_sb[:], None,
                                            op0=Alu.add)
                    nc.sync.dma_start(out=outT[:, t * PT:(t + 1) * PT],
                                      in_=ot[0:3, :])

            if not last and DBG != 1:
                nc.gpsimd.collective_compute(
                    "AllGather", Alu.bypass,
                    replica_groups=[list(range(NCORES))],
                    ins=[xdst.ap().opt()],
                    outs=[xfull[li].ap().opt()],
                )

        layer1()
        if DBG != 1:
            conv_layer(1, xfull[0], xloc[1])
            conv_layer(2, xfull[1], None)

    nc.compile()
    return nc


# ---------------------------------------------------------------- host prep
def _layout_per_core(V):
    """[PPCP, K] -> [PT, COLS] with out[q*32+k, t*32+b] = V[t*96+b*3+q, k].

    Point r (in-tile) sits at slot (q=r%3, b=r//3) so consecutive r rotate
    PE row groups (LDWEIGHTS pull-ahead)."""
    if IROT == 0:
        return (V.reshape(NTILES, 4, 32, K)
                .transpose(1, 3, 0, 2)
                .reshape(PT, COLS))
    return (V.reshape(NTILES, 32, 4, K)   # [t, b, q, k]
            .transpose(2, 3, 0, 1)        # [q, k, t, b]
            .reshape(PT, COLS))


def _ball_to_cube_np(u):
    x, y, z = u[..., 0], u[..., 1], u[..., 2]
    sq = x * x + y * y + z * z
    norm = np.sqrt(np.maximum(sq, EPS))
    rho2 = x * x + y * y
    in_cone = 1.25 * z * z > rho2
    s1 = np.sqrt(3.0 * norm / (norm + np.abs(z)))
    with np.errstate(divide="ignore", invalid="ignore"):
        s2 = norm / np.sqrt(np.maximum(rho2, EPS))
    s = np.where(in_cone, s1, s2)
    xc = x * s
    yc = y * s
    zc = np.where(in_cone, np.sign(z) * norm, 1.5 * z)
    tiny = sq < EPS
    xc = np.where(tiny, 0.0, xc)
    yc = np.where(tiny, 0.0, yc)
    zc = np.where(tiny, 0.0, zc)
    sq_xy = xc * xc + yc * yc
    norm_xy = np.sqrt(np.maximum(sq_xy, EPS))
    x_major = np.abs(yc) <= np.abs(xc)
    xd = np.where(np.abs(xc) < EPS, 1.0, xc)
    yd = np.where(np.abs(yc) < EPS, 1.0, yc)
    tx = np.sign(xc) * norm_xy
    ty = np.sign(yc) * norm_xy
    xq = np.where(x_major, tx, ty * FOUR_OVER_PI * np.arctan(xc / yd))
    yq = np.where(x_major, tx * FOUR_OVER_PI * np.arctan(yc / xd), ty)
    tiny_xy = sq_xy < EPS
    xq = np.where(tiny_xy, 0.0, xq)
    yq = np.where(tiny_xy, 0.0, yq)
    return np.stack([xq, yq, zc], axis=-1)


def _host_a1(u_raw, nidx, nmask, f4):
    """A1[j, m, c] = sum_k S[j,k,m] f4[nidx[j,k], c] (trilinear scatter)."""
    n = u_raw.shape[0]
    coords = (_ball_to_cube_np(u_raw) + 1.0) * np.float32(0.5 * (KS - 1))
    i0f = np.floor(coords)
    frac = coords - i0f
    i0 = np.clip(i0f.astype(np.int64), 0, KS - 1)
    i1 = np.minimum(i0 + 1, KS - 1)
    maskf = nmask.astype(np.float32)
    fN = f4[nidx]                                     # [N, K, 4]
    base = (np.arange(n, dtype=np.int64)[:, None] * M)
    A1 = np.zeros((n * M, 4), np.float32)
    for cx in (0, 1):
        ix = i1[..., 0] if cx else i0[..., 0]
        wx = frac[..., 0] if cx else 1.0 - frac[..., 0]
        for cy in (0, 1):
            iy = i1[..., 1] if cy else i0[..., 1]
            wy = frac[..., 1] if cy else 1.0 - frac[..., 1]
            for cz in (0, 1):
                iz = i1[..., 2] if cz else i0[..., 2]
                wz = frac[..., 2] if cz else 1.0 - frac[..., 2]
                w = (wx * wy * wz * maskf)
                flat = (base + (ix * KS + iy) * KS + iz).ravel()
                for c in range(4):
                    A1[:, c] += np.bincount(
                        flat, weights=(w * fN[..., c]).ravel(),
                        minlength=n * M)
    return A1.reshape(n, M, 4)


def _prep_inputs(feats, pos, neighbor_idx, neighbor_mask,
                 W1, b1, W2, b2, W3, b3,
                 Wfc1, bfc1, Wfc2, bfc2, Wfc3, bfc3, Wout, bout):
    import ml_dtypes
    bf16 = ml_dtypes.bfloat16
    f4 = np.asarray(feats, np.float32)
    pos = np.asarray(pos, np.float32)
    nidx = np.asarray(neighbor_idx, np.int32)
    nmask = np.asarray(neighbor_mask, bool)

    # u (masked -> BIG), cnt_inv
    u_raw = (pos[nidx] - pos[:, None, :]) * np.float32(2.0 / EXTENT)
    A1 = _host_a1(u_raw.astype(np.float32), nidx, nmask, f4)
    u = np.where(nmask[..., None], u_raw, np.float32(BIG)).astype(np.float32)
    cnt = nmask.sum(axis=1)
    cnt_inv = (1.0 / np.maximum(cnt, 1)).astype(np.float32)

    # global index -> padded allgather row
    g = nidx.astype(np.int64)
    remap = ((g // PPC) * PPCP + (g % PPC)).astype(np.int32)

    def warr(W, cin, cout):
        Wp = np.zeros((M, C, C), np.float32)
        Wp[:, :cin, :cout] = np.asarray(W, np.float32).reshape(M, cin, cout)
        return (Wp.reshape(HC, 2, C, C).transpose(1, 2, 0, 3)
                .reshape(128, HC * C).astype(bf16).copy())

    w2 = warr(W2, 64, 64)
    w3 = warr(W3, 64, 32)

    # layer-1 packed W1: [cb*4+ch, k7*64+o] = W1[k7*32+cb, ch, o]
    W1a = np.asarray(W1, np.float32).reshape(M, 4, 64)
    W1p = np.zeros((224, 4, 64), np.float32)
    W1p[:M] = W1a
    w1 = (W1p.reshape(7, 32, 4, 64).transpose(1, 2, 0, 3)
          .reshape(128, 7 * 64).astype(bf16).copy())

    def btile(b, n):
        bp = np.zeros(C, np.float32)
        bp[:n] = np.asarray(b, np.float32)
        return np.tile(bp, (PT, 1)).copy()

    bias1, bias2, bias3 = btile(b1, 64), btile(b2, 64), btile(b3, 32)
    iota6 = np.tile(np.arange(6, dtype=np.float32), (PT, 1)).copy()

    wfc1 = np.zeros((64, 64), bf16)
    wfc1[:32, :] = np.asarray(Wfc1, np.float32)
    wfc2 = np.asarray(Wfc2, np.float32).astype(bf16).copy()
    wfc3 = np.zeros((64, 64), bf16)
    wfc3[:, :32] = np.asarray(Wfc3, np.float32)
    wout = np.zeros((64, 8), bf16)
    wout[:32, :3] = np.asarray(Wout, np.float32)

    def bcol(b, n, p):
        v = np.zeros((p, 1), np.float32)
        v[:n, 0] = np.asarray(b, np.float32)
        return v

    bfc1c, bfc2c, bfc3c = bcol(bfc1, 64, 64), bcol(bfc2, 64, 64), \
        bcol(bfc3, 32, 64)
    boutc = bcol(bout, 3, 8)

    in_maps = []
    for c in range(NCORES):
        # per-core padded [PPCP, K] views
        uloc = np.full((PPCP, K, 3), BIG, np.float32)
        uloc[:PPC] = u[c * PPC:(c + 1) * PPC]
        nloc = np.zeros((PPCP, K), np.int32)
        nloc[:PPC] = remap[c * PPC:(c + 1) * PPC]
        cloc = np.ones(PPCP, np.float32)
        cloc[:PPC] = cnt_inv[c * PPC:(c + 1) * PPC]
        # layer-1 A1, K-packed: a1[cb*4+ch, (t*7+k7)*PT+r]
        a1c = np.zeros((PPCP, 224, 4), np.float32)
        a1c[:PPC, :M] = A1[c * PPC:(c + 1) * PPC]
        a1c_dev = (a1c.reshape(NTILES, PT, 7, 32, 4)
                   .transpose(3, 4, 0, 2, 1)
                   .reshape(128, NTILES * 7 * PT).astype(bf16).copy())

        uin = np.concatenate(
            [_layout_per_core(uloc[:, :, d]) for d in range(3)],
            axis=1).astype(np.float32).copy()
        nidx_dev = np.zeros((128, COLS), np.int32)
        nidx_dev[:PT] = _layout_per_core(nloc).astype(np.int32)
        # int16 dma_gather index layout: per tile t, flat[b*128+p] =
        # nidx_dev[p, t*32+b]; [16, NI/16] wrap idx16[q, cc] = flat[cc*16+q],
        # replicated x8 across partition groups
        n16 = []
        for t in range(NTILES):
            flat = nidx_dev[:, t * 32:(t + 1) * 32].T.reshape(-1)
            idx16 = flat.reshape(NI // 16, 16).T
            n16.append(np.tile(idx16, (8, 1)))
        nidx16 = np.concatenate(n16, axis=1).astype(np.int16).copy()
        cnti = cloc.reshape(NTILES, PT).T.astype(np.float32).copy()

        in_maps.append({
            "nidx": nidx16, "uin": uin, "cntinv": cnti,
            "a1": a1c_dev, "w1": w1, "w2": w2, "w3": w3,
            "bias1": bias1, "bias2": bias2, "bias3": bias3,
            "iota6": iota6,
            "wfc1": wfc1, "wfc2": wfc2, "wfc3": wfc3, "wout": wout,
            "bfc1": bfc1c, "bfc2": bfc2c, "bfc3": bfc3c, "bout": boutc,
        })
    return in_maps


def _run(in_maps, trace=False, **kw):
    from concourse.bass_utils import run_bass_kernel_spmd
    if "nc" not in _CACHE:
        _CACHE["nc"] = _build_program()
    nc = _CACHE["nc"]
    res = run_bass_kernel_spmd(nc, in_maps, core_ids=list(range(NCORES)),
                               trace=trace, **kw)
    return res


def kernel(**inputs):
    in_maps = _prep_inputs(**{k: np.asarray(v) for k, v in inputs.items()})
    res = _run(in_maps)
    outs = []
    for c in range(NCORES):
        oc = res.results[c]["outT"]          # [3, PPCP]
        outs.append(oc[:, :PPC].T)           # [PPC, 3]
    return np.concatenate(outs, axis=0).astype(np.float32)

